# revision 1
# baseline (speedup 1.0000x reference)
"""GGNN message passing + bilinear readout on 8 TRN2 NeuronCores.

Problem: nn_BaselineModel_36687610642509 (gnn_message_passing).

reference:
    for 8 iters:  per_edge = einsum('sd,edh->seh', h, W_msg)
                  messages = einsum('ste,seh->th', edge, per_edge) + b_msg
                  h = GRU(h, messages)          (Wi, Wh, b_gru)
    logits = einsum('id,de,je->ij', h, A_readout, h)

Distribution (1D node parallelism, 8 cores, SENDER-sharded):
    core k owns nodes s_k = [256k, 256k+256).
    - edge shard edge[s_k, :, :] lives in SBUF for the whole kernel (bf16, 8 MiB).
    - h is sharded; each core computes per_edge for its own senders only,
      then partial messages for ALL destinations:
          msgsT_partial[d, t] = sum_e  pe_e[s_k, d]^T-contracted edge_e[s_k, t]
      One ReduceScatter(add) per iteration sums partials across cores and
      hands core k its own destination shard (dst shard == sender shard).
    - GRU update runs shard-locally; no other communication per iteration.
    - One final AllGather of h feeds the pairwise bilinear readout; each
      core emits its 256 rows of the [2048, 2048] logits.

Everything on-chip is kept in transposed [dim, node] layout so every matmul
contracts over the partition axis with zero transposes anywhere.
Matmul operands are bf16 (PSUM accumulation fp32).
"""

import sys

for _p in ("/opt/trn_rl_repo",):
    if _p not in sys.path:
        sys.path.insert(0, _p)

import numpy as np
import ml_dtypes

import concourse.bacc as bacc
import concourse.tile as tile
import concourse.mybir as mybir
from concourse import bass_utils

dt = mybir.dt
AF = mybir.ActivationFunctionType

N_CORES = 8
N = 2048          # nodes
D = 128           # embedding dim
E = 8             # edge channels
ITERS = 8
S = N // N_CORES  # 256 nodes per core
RG = [list(range(N_CORES))]


def build_nc(reps=1, wire_bf16=False, skip_coll=False, a2a=False, wide=False,
             rdma=False):
    """rdma=True replaces collective_compute with direct SBUF->SBUF
    remote_dma_broadcast exchange. Requires XOR-permuted edge shards host-side
    (make_in_maps(xor_perm=True)) and XOR un-permutation of output columns.
    Broadcast j reads fixed stage block j (partial messages for absolute
    destination core own^j) and writes fixed recv slot j on core own^j; the
    receiver sums all 8 slots, so the sender-dependent slot order is
    irrelevant for the messages, and the readout's permuted logits columns
    are un-permuted on the host."""
    nc = bacc.Bacc("TRN2", target_bir_lowering=False, debug=False,
                   num_devices=N_CORES)

    edgek = nc.dram_tensor("edgek", [E * S, N], dt.bfloat16, kind="ExternalInput")
    h0t = nc.dram_tensor("h0t", [D, S], dt.bfloat16, kind="ExternalInput")
    wmsg = nc.dram_tensor("wmsg", [D, E * D], dt.bfloat16, kind="ExternalInput")
    wi = nc.dram_tensor("wi", [D, 3 * D], dt.bfloat16, kind="ExternalInput")
    wh = nc.dram_tensor("wh", [D, 3 * D], dt.bfloat16, kind="ExternalInput")
    bias = nc.dram_tensor("bias", [D, 3], dt.float32, kind="ExternalInput")
    aro = nc.dram_tensor("aro", [D, D], dt.bfloat16, kind="ExternalInput")
    out = nc.dram_tensor("out", [S, N], dt.float32, kind="ExternalOutput")

    wdt = dt.bfloat16 if wire_bf16 else dt.float32

    if rdma:
        stage_sb = nc.alloc_sbuf_tensor("stage_sb", [D, N], dt.bfloat16)
        recv_sb = [nc.alloc_sbuf_tensor(f"recv_sb{p}", [D, N], dt.bfloat16)
                   for p in range(2)]
        hsend_sb = nc.alloc_sbuf_tensor("hsend_sb", [D, S], dt.bfloat16)
        htf_sb = nc.alloc_sbuf_tensor("htf_sb", [D, N], dt.bfloat16)
        rsem = nc.alloc_semaphore(name="rsem")
        lsem = nc.alloc_semaphore(name="lsem")
        psem = nc.alloc_semaphore(name="psem")
        rounds = [0]  # global exchange-round counter across reps
        preps = [0]   # global prep counter

    with tile.TileContext(nc) as tc:
        with (
            tc.tile_pool(name="const", bufs=1) as cpool,
            tc.tile_pool(name="sb", bufs=2) as spool,
            tc.tile_pool(name="stage", bufs=4) as stpool,
            tc.tile_pool(name="pe_ps", bufs=2, space="PSUM") as pe_ps,
            tc.tile_pool(name="mm_ps", bufs=3, space="PSUM") as mm_ps,
            tc.tile_pool(name="gru_ps", bufs=3, space="PSUM") as gru_ps,
            tc.tile_pool(name="dram", bufs=2, space="DRAM") as dram,
        ):
            for rep in range(reps):
                # ---- load constants (edge shard stays resident all kernel) ----
                edge_sb = {}
                for e in range(E):
                    for ss in range(2):
                        t = cpool.tile([D, N], dt.bfloat16, tag=f"edge{e}_{ss}")
                        r0 = e * S + ss * D
                        nc.sync.dma_start(t[:], edgek.ap()[r0:r0 + D, :])
                        edge_sb[(e, ss)] = t
                wmsg_sb = cpool.tile([D, E * D], dt.bfloat16, tag="wmsg")
                nc.sync.dma_start(wmsg_sb[:], wmsg.ap())
                wi_sb = cpool.tile([D, 3 * D], dt.bfloat16, tag="wi")
                nc.sync.dma_start(wi_sb[:], wi.ap())
                wh_sb = cpool.tile([D, 3 * D], dt.bfloat16, tag="wh")
                nc.sync.dma_start(wh_sb[:], wh.ap())
                bias_sb = cpool.tile([D, 3], dt.float32, tag="bias")
                nc.sync.dma_start(bias_sb[:], bias.ap())
                aro_sb = cpool.tile([D, D], dt.bfloat16, tag="aro")
                nc.sync.dma_start(aro_sb[:], aro.ap())

                hT = spool.tile([D, S], dt.bfloat16, tag="hT")
                nc.sync.dma_start(hT[:], h0t.ap())

                if rdma == "hoist":
                    # generate ALL descriptor preps for this rep upfront
                    # (descriptors encode only static addresses); rounds then
                    # cost just trigger_dma + sem waits.
                    with tc.tile_critical(name=f"prep{rep}"):
                        g = nc.gpsimd
                        for r in range(ITERS + 1):
                            rv = recv_sb[(rounds[0] + r) % 2]
                            for j in range(N_CORES):
                                dests = [None] * N_CORES
                                dests[j] = (0, j)
                                if r < ITERS:
                                    out_ap = rv.ap()[:, j * S:(j + 1) * S]
                                    in_ap = stage_sb.ap()[:, j * S:(j + 1) * S]
                                else:
                                    out_ap = htf_sb.ap()[:, j * S:(j + 1) * S]
                                    in_ap = hsend_sb.ap()
                                g.remote_dma_broadcast(
                                    out_ap, in_ap, rsem, lsem, rdests=dests,
                                ).then_inc(psem, 1)
                                preps[0] += 1
                        g.wait_ge(psem, preps[0])

                for it in range(ITERS):
                    # per_edge[s, (e,h)] = h_k @ [W_0 | ... | W_7]
                    pe_bf = []
                    for ss in range(2):
                        pb = spool.tile([D, E * D], dt.bfloat16, tag=f"pe{ss}")
                        for half in range(2):
                            pp = pe_ps.tile([D, 512], dt.float32, tag="pe_ps")
                            nc.tensor.matmul(
                                pp[:],
                                hT[:, ss * D:(ss + 1) * D],
                                wmsg_sb[:, half * 512:(half + 1) * 512],
                                start=True, stop=True,
                            )
                            nc.vector.tensor_copy(
                                pb[:, half * 512:(half + 1) * 512], pp[:])
                        pe_bf.append(pb)

                    # partial messages for every destination shard, then RS
                    if not rdma:
                        rsin = dram.tile([N_CORES * D, S], wdt, tag="rsin")
                    CW = 2 * S if wide else S  # big-mm moving width
                    for c in range(N * 1 // CW):
                        mp = mm_ps.tile([D, CW], dt.float32, tag="mm")
                        q = 0
                        for e in range(E):
                            for ss in range(2):
                                nc.tensor.matmul(
                                    mp[:],
                                    pe_bf[ss][:, e * D:(e + 1) * D],
                                    edge_sb[(e, ss)][:, c * CW:(c + 1) * CW],
                                    start=(q == 0), stop=(q == 15),
                                )
                                q += 1
                        if rdma:
                            nc.vector.tensor_copy(
                                stage_sb.ap()[:, c * CW:(c + 1) * CW], mp[:])
                        else:
                            st = stpool.tile([D, CW], wdt, tag="mmstage")
                            nc.vector.tensor_copy(st[:], mp[:])
                            for b in range(CW // S):
                                j = c * (CW // S) + b
                                nc.sync.dma_start(rsin[j * D:(j + 1) * D, :],
                                                  st[:, b * S:(b + 1) * S])

                    if rdma:
                        rv = recv_sb[rounds[0] % 2]
                        with tc.tile_critical(name=f"xch{rounds[0]}",
                                              no_gpsimd_drain=False):
                            g = nc.gpsimd
                            if rdma != "hoist":
                                for j in range(N_CORES):
                                    dests = [None] * N_CORES
                                    dests[j] = (0, j)
                                    prep = g.remote_dma_broadcast(
                                        rv.ap()[:, j * S:(j + 1) * S],
                                        stage_sb.ap()[:, j * S:(j + 1) * S],
                                        rsem, lsem, rdests=dests,
                                    )
                                    prep.then_inc(psem, 1)
                                    preps[0] += 1
                                g.wait_ge(psem, preps[0])
                            g.trigger_dma(N_CORES)
                            rounds[0] += 1
                            g.wait_ge(rsem, 16 * rounds[0])
                            g.wait_ge(lsem, 16 * N_CORES * rounds[0])
                        acc0 = stpool.tile([D, S], dt.float32, tag="acc0")
                        nc.vector.tensor_add(acc0[:], rv.ap()[:, 0:S],
                                             rv.ap()[:, S:2 * S])
                        acc1 = stpool.tile([D, S], dt.float32, tag="acc1")
                        nc.vector.tensor_add(acc1[:], rv.ap()[:, 2 * S:3 * S],
                                             rv.ap()[:, 3 * S:4 * S])
                        acc2 = stpool.tile([D, S], dt.float32, tag="acc2")
                        nc.vector.tensor_add(acc2[:], rv.ap()[:, 4 * S:5 * S],
                                             rv.ap()[:, 5 * S:6 * S])
                        acc3 = stpool.tile([D, S], dt.float32, tag="acc3")
                        nc.vector.tensor_add(acc3[:], rv.ap()[:, 6 * S:7 * S],
                                             rv.ap()[:, 7 * S:8 * S])
                        nc.vector.tensor_add(acc0[:], acc0[:], acc1[:])
                        nc.vector.tensor_add(acc2[:], acc2[:], acc3[:])
                        msgs_bf = spool.tile([D, S], dt.bfloat16, tag="msgsbf")
                        nc.vector.tensor_add(msgs_bf[:], acc0[:], acc2[:])
                    elif a2a:
                        a2out = dram.tile([N_CORES * D, S], wdt, tag="a2out")
                        if not skip_coll:
                            nc.gpsimd.collective_compute(
                                "AllToAll", mybir.AluOpType.bypass,
                                replica_groups=RG,
                                ins=[rsin.opt()], outs=[a2out.opt()],
                            )
                        else:
                            a2out = rsin
                        # local 8-way sum of the received partials
                        parts = spool.tile([D, N_CORES * S], wdt, tag="parts")
                        for j in range(N_CORES):
                            nc.sync.dma_start(parts[:, j * S:(j + 1) * S],
                                              a2out[j * D:(j + 1) * D, :])
                        acc = []
                        for l in range(2):
                            acc_t = spool.tile([D, S], dt.float32, tag=f"acc{l}")
                            acc.append(acc_t)
                        nc.vector.tensor_add(acc[0][:], parts[:, 0:S],
                                             parts[:, S:2 * S])
                        for j in range(2, N_CORES - 1):
                            nc.vector.tensor_add(acc[j % 2][:],
                                                 acc[(j + 1) % 2][:],
                                                 parts[:, j * S:(j + 1) * S])
                        msgs_bf = spool.tile([D, S], dt.bfloat16, tag="msgsbf")
                        nc.vector.tensor_add(msgs_bf[:], acc[1][:],
                                             parts[:, 7 * S:8 * S])
                    else:
                        rsout = dram.tile([D, S], wdt, tag="rsout")
                        if not skip_coll:
                            nc.gpsimd.collective_compute(
                                "ReduceScatter", mybir.AluOpType.add,
                                replica_groups=RG,
                                ins=[rsin.opt()], outs=[rsout.opt()],
                            )
                            msgs_src = rsout
                        else:
                            msgs_src = rsin[0:D, :]
                        msgs = spool.tile([D, S], wdt, tag="msgs")
                        nc.sync.dma_start(msgs[:], msgs_src[:] if msgs_src is rsout else msgs_src)
                        if wire_bf16:
                            msgs_bf = msgs
                        else:
                            msgs_bf = spool.tile([D, S], dt.bfloat16, tag="msgsbf")
                            nc.vector.tensor_copy(msgs_bf[:], msgs[:])

                    # GRU: r/z gates via PSUM-accumulated gi+gh, biases folded
                    new_hT = spool.tile([D, S], dt.bfloat16, tag="hT")
                    gate = []
                    for g in range(2):
                        gp = gru_ps.tile([D, S], dt.float32, tag="gru")
                        nc.tensor.matmul(gp[:], wi_sb[:, g * D:(g + 1) * D],
                                         msgs_bf[:], start=True, stop=False)
                        nc.tensor.matmul(gp[:], wh_sb[:, g * D:(g + 1) * D],
                                         hT[:], start=False, stop=True)
                        gs = stpool.tile([D, S], dt.float32, tag=f"g{g}")
                        nc.scalar.activation(gs[:], gp[:], AF.Sigmoid,
                                             bias=bias_sb[:, g:g + 1])
                        gate.append(gs)
                    r_g, z_g = gate

                    inp = gru_ps.tile([D, S], dt.float32, tag="gru")
                    nc.tensor.matmul(inp[:], wi_sb[:, 2 * D:3 * D], msgs_bf[:],
                                     start=True, stop=True)
                    hnp = gru_ps.tile([D, S], dt.float32, tag="gru")
                    nc.tensor.matmul(hnp[:], wh_sb[:, 2 * D:3 * D], hT[:],
                                     start=True, stop=True)
                    t1 = stpool.tile([D, S], dt.float32, tag="t1")
                    nc.vector.tensor_mul(t1[:], r_g[:], hnp[:])
                    t2 = stpool.tile([D, S], dt.float32, tag="t2")
                    nc.vector.tensor_add(t2[:], t1[:], inp[:])
                    n_sb = stpool.tile([D, S], dt.float32, tag="n")
                    nc.scalar.activation(n_sb[:], t2[:], AF.Tanh,
                                         bias=bias_sb[:, 2:3])
                    # h_new = n + z * (h - n)
                    d1 = stpool.tile([D, S], dt.float32, tag="d1")
                    nc.vector.tensor_sub(d1[:], hT[:], n_sb[:])
                    d2 = stpool.tile([D, S], dt.float32, tag="d2")
                    nc.vector.tensor_mul(d2[:], z_g[:], d1[:])
                    nc.vector.tensor_add(new_hT[:], n_sb[:], d2[:])
                    hT = new_hT

                # ---- readout: logits rows = (h_k A) @ h^T ----
                if rdma:
                    nc.vector.tensor_copy(hsend_sb.ap(), hT[:])
                    with tc.tile_critical(name=f"hag{rounds[0]}"):
                        g = nc.gpsimd
                        if rdma != "hoist":
                            for j in range(N_CORES):
                                dests = [None] * N_CORES
                                dests[j] = (0, j)
                                prep = g.remote_dma_broadcast(
                                    htf_sb.ap()[:, j * S:(j + 1) * S],
                                    hsend_sb.ap(),
                                    rsem, lsem, rdests=dests,
                                )
                                prep.then_inc(psem, 1)
                                preps[0] += 1
                            g.wait_ge(psem, preps[0])
                        g.trigger_dma(N_CORES)
                        rounds[0] += 1
                        g.wait_ge(rsem, 16 * rounds[0])
                        g.wait_ge(lsem, 16 * N_CORES * rounds[0])
                    hTf = htf_sb.ap()
                else:
                    agin = dram.tile([D, S], dt.bfloat16, tag="agin")
                    nc.sync.dma_start(agin[:], hT[:])
                    agout = dram.tile([N_CORES * D, S], dt.bfloat16, tag="agout")
                    if not skip_coll:
                        nc.gpsimd.collective_compute(
                            "AllGather", mybir.AluOpType.bypass,
                            replica_groups=RG,
                            ins=[agin.opt()], outs=[agout.opt()],
                        )
                    hTf = spool.tile([D, N], dt.bfloat16, tag="hTf")
                    for j in range(N_CORES):
                        src = agout[j * D:(j + 1) * D, :] if not skip_coll else agin[:]
                        nc.sync.dma_start(hTf[:, j * S:(j + 1) * S], src)

                hap = mm_ps.tile([D, S], dt.float32, tag="mm")
                nc.tensor.matmul(hap[:], aro_sb[:], hT[:], start=True, stop=True)
                hA_bf = spool.tile([D, S], dt.bfloat16, tag="hA")
                nc.vector.tensor_copy(hA_bf[:], hap[:])

                for isub in range(2):
                    for jc in range(N_CORES):
                        lp = mm_ps.tile([D, S], dt.float32, tag="mm")
                        nc.tensor.matmul(lp[:],
                                         hA_bf[:, isub * D:(isub + 1) * D],
                                         hTf[:, jc * S:(jc + 1) * S],
                                         start=True, stop=True)
                        ost = stpool.tile([D, S], dt.float32, tag="ost")
                        nc.vector.tensor_copy(ost[:], lp[:])
                        nc.sync.dma_start(
                            out.ap()[isub * D:(isub + 1) * D,
                                     jc * S:(jc + 1) * S],
                            ost[:])

    nc.compile()
    return nc


def build_probe_nc():
    """Tiny kernel: each core broadcasts its rank via the same per-slot
    remote_dma_broadcast pattern as the main kernel; receivers report which
    rank landed in each slot. Yields PEER[k][j] = rank reached by core k's
    slot-j send (XOR-symmetric, so also the rank whose slot-j data core k
    receives)."""
    nc = bacc.Bacc("TRN2", target_bir_lowering=False, debug=False,
                   num_devices=N_CORES)
    x = nc.dram_tensor("x", [1, 8], dt.float32, kind="ExternalInput")
    y = nc.dram_tensor("y", [8, 8], dt.float32, kind="ExternalOutput")
    src = nc.alloc_sbuf_tensor("src_sb", [128, 8], dt.float32)
    recv = nc.alloc_sbuf_tensor("recv_sb", [128, 64], dt.float32)
    rsem = nc.alloc_semaphore(name="rsem")
    lsem = nc.alloc_semaphore(name="lsem")
    psem = nc.alloc_semaphore(name="psem")
    with tile.TileContext(nc) as tc:
        with tc.tile_pool(name="sb", bufs=1) as sb:
            t = sb.tile([1, 8], dt.float32, tag="t")
            nc.sync.dma_start(t[:], x.ap())
            nc.vector.tensor_copy(src.ap()[0:1, :], t[:])
            with tc.tile_critical(name="probe"):
                g = nc.gpsimd
                for j in range(N_CORES):
                    dests = [None] * N_CORES
                    dests[j] = (0, j)
                    g.remote_dma_broadcast(
                        recv.ap()[:, j * 8:(j + 1) * 8], src.ap(),
                        rsem, lsem, rdests=dests).then_inc(psem, 1)
                g.wait_ge(psem, 8)
                g.trigger_dma(8)
                g.wait_ge(rsem, 16)
                g.wait_ge(lsem, 128)
            o2 = sb.tile([1, 64], dt.float32, tag="o2")
            nc.vector.tensor_copy(o2[:], recv.ap()[0:1, :])
            nc.sync.dma_start(y.ap().rearrange("a b -> (a b)")[None, :], o2[:])
    nc.compile()
    return nc


def get_peer_map():
    if "peer" in _cache:
        return _cache["peer"]
    nc = build_probe_nc()
    in_maps = [{"x": np.full((1, 8), k, np.float32)} for k in range(N_CORES)]
    res = bass_utils.run_bass_kernel_spmd(
        nc, in_maps, core_ids=list(range(N_CORES)))
    peer = np.zeros((N_CORES, N_CORES), np.int64)
    for k in range(N_CORES):
        peer[k] = res.results[k]["y"].reshape(-1)[::8].astype(np.int64)
    # sanity: each row must be a permutation with peer[k][0] == k
    for k in range(N_CORES):
        assert sorted(peer[k]) == list(range(N_CORES)) and peer[k][0] == k, peer
        for j in range(N_CORES):
            assert peer[peer[k][j]][j] == k, peer  # XOR symmetry
    _cache["peer"] = peer
    return peer


def make_in_maps(node_embeddings, edge_embeddings, W_msg, b_msg, Wi, Wh,
                 b_gru, A_readout, xor_perm=False, peer=None):
    bf16 = ml_dtypes.bfloat16
    wmsg = np.ascontiguousarray(
        W_msg.transpose(1, 0, 2).reshape(D, E * D)).astype(bf16)
    wi_b = np.ascontiguousarray(Wi).astype(bf16)
    wh_b = np.ascontiguousarray(Wh).astype(bf16)
    # messages enter the GRU only through  gi = (raw_msgs + b_msg) @ Wi + b_gru,
    # so fold b_msg into a per-gate bias (fp32, exact).
    b_eff = (b_msg.astype(np.float64) @ Wi.astype(np.float64)
             + b_gru.astype(np.float64)).astype(np.float32)
    bias = np.ascontiguousarray(b_eff.reshape(3, D).T)  # [D, 3]
    aro_b = np.ascontiguousarray(A_readout).astype(bf16)

    in_maps = []
    for k in range(N_CORES):
        sl = slice(k * S, (k + 1) * S)
        ek = np.ascontiguousarray(
            edge_embeddings[sl].transpose(2, 0, 1).reshape(E * S, N)
        ).astype(bf16)
        if xor_perm:
            cols = np.concatenate(
                [np.arange(peer[k][j] * S, (peer[k][j] + 1) * S)
                 for j in range(N_CORES)])
            ek = np.ascontiguousarray(ek[:, cols])
        h0t = np.ascontiguousarray(node_embeddings[sl].T).astype(bf16)
        in_maps.append({
            "edgek": ek, "h0t": h0t, "wmsg": wmsg, "wi": wi_b, "wh": wh_b,
            "bias": bias, "aro": aro_b,
        })
    return in_maps


_cache = {}


# The remote_dma_broadcast exchange (USE_RDMA=True) is functionally correct
# (rel err 8.0e-3, peer routing probed at runtime) but measured 794us vs the
# collective_compute version's 405us: the 9 tile_critical sections' engine
# drains + serialized SWDGE desc-gen cost more than the ~30us/collective ncfw
# floor they avoid. Keeping the collective path as default.
USE_RDMA = False


def unpermute_out(results, peer):
    """Undo the peer-slot column permutation of the rdma kernel's outputs."""
    logits = np.empty((N, N), np.float32)
    for k in range(N_CORES):
        ok = results[k]["out"]
        for j in range(N_CORES):
            a = peer[k][j]
            logits[k * S:(k + 1) * S, a * S:(a + 1) * S] = \
                ok[:, j * S:(j + 1) * S]
    return logits


def kernel(node_embeddings, edge_embeddings, W_msg, b_msg, Wi, Wh, b_gru,
           A_readout):
    peer = get_peer_map() if USE_RDMA else None
    if "nc" not in _cache:
        _cache["nc"] = build_nc(reps=1, wire_bf16=True, wide=True,
                                rdma=USE_RDMA)
    nc = _cache["nc"]
    in_maps = make_in_maps(node_embeddings, edge_embeddings, W_msg, b_msg,
                           Wi, Wh, b_gru, A_readout, xor_perm=USE_RDMA,
                           peer=peer)
    res = bass_utils.run_bass_kernel_spmd(
        nc, in_maps, core_ids=list(range(N_CORES)))
    if USE_RDMA:
        return unpermute_out(res.results, peer)
    return np.concatenate([res.results[k]["out"] for k in range(N_CORES)],
                          axis=0)



# revision 2
# speedup vs baseline: 1.0918x; 1.0918x over previous
"""GGNN message passing + bilinear readout on 8 TRN2 NeuronCores (v3b: destination-sharded + AllGather).

Destination-sharded 1D node parallelism with the "agg" contraction order:

  core k owns destinations t_k = [256k, 256k+256).
  - edge shard edge[:, t_k, :] resident in SBUF as 8(e) x 16(s-chunk)
    tiles of [128 senders, 256 dst] bf16 (8 MiB).
  - per iter: all-gather h (64 KB per core, node-rows [N, D]) — the
    cheapest collective primitive (~5 us at 8 cores) — then locally:
        agg_e[d, t]  = sum_s h[s, d] * edge_e[s, t]      (128 MMs, PSUM acc)
        msgsT[h, t]  = sum_e W_e[d, h]^T-contracted agg_e  (8 MMs)
        h_new        = GRU(msgsT, hT_own)                  (as v1)
    and 2 PE transposes turn h_new^T into node-rows for the next AG.
  - h0 is supplied replicated host-side, so only 7 iteration AGs + 1
    readout AG run. The readout AG output [N, D] is DMA-transposed
    (X-bar) into hT-full [D, N] for the bilinear logits rows.

No collective ReduceScatter of 512 KB partials (v1: ~21 us/round), no
remote-DMA descriptor generation (~10 us/prep on Q7), no critical
sections — everything stays on Tile's scheduler so DMA/collective/compute
overlap freely.
"""

import sys

for _p in ("/opt/trn_rl_repo",):
    if _p not in sys.path:
        sys.path.insert(0, _p)

import numpy as np
import ml_dtypes

import concourse.bacc as bacc
import concourse.tile as tile
import concourse.mybir as mybir
from concourse import bass_utils

dt = mybir.dt
AF = mybir.ActivationFunctionType

N_CORES = 8
N = 2048
D = 128
E = 8
ITERS = 8
S = N // N_CORES  # 256
NCHUNK = N // D   # 16 sender chunks
RG = [list(range(N_CORES))]


def build_nc(reps=1, skip_coll=False):
    nc = bacc.Bacc("TRN2", target_bir_lowering=False, debug=False,
                   num_devices=N_CORES)

    # edge shard, sender-chunk-major: [(e, s_chunk) x 128, t_own]
    edgek = nc.dram_tensor("edgek", [E // 2 * N, 2 * S], dt.bfloat16,
                           kind="ExternalInput")
    h0t = nc.dram_tensor("h0t", [D, S], dt.bfloat16, kind="ExternalInput")
    h0r = nc.dram_tensor("h0r", [N, D], dt.bfloat16, kind="ExternalInput")
    wmsg = nc.dram_tensor("wmsg", [D, E * D], dt.bfloat16, kind="ExternalInput")
    wi = nc.dram_tensor("wi", [D, 3 * D], dt.bfloat16, kind="ExternalInput")
    wh = nc.dram_tensor("wh", [D, 3 * D], dt.bfloat16, kind="ExternalInput")
    bias = nc.dram_tensor("bias", [D, 3], dt.float32, kind="ExternalInput")
    aro = nc.dram_tensor("aro", [D, D], dt.bfloat16, kind="ExternalInput")
    ident = nc.dram_tensor("ident", [D, D], dt.bfloat16, kind="ExternalInput")
    out = nc.dram_tensor("out", [S, N], dt.float32, kind="ExternalOutput")

    with tile.TileContext(nc) as tc:
        with (
            tc.tile_pool(name="const", bufs=1) as cpool,
            tc.tile_pool(name="sb", bufs=2) as spool,
            tc.tile_pool(name="stage", bufs=2) as stpool,
            tc.tile_pool(name="agg_ps", bufs=1, space="PSUM") as agg_ps,
            tc.tile_pool(name="gru_ps", bufs=4, space="PSUM") as gru_ps,
            tc.tile_pool(name="dram", bufs=2, space="DRAM") as dram,
        ):
            for rep in range(reps):
                # ---- resident constants ----
                edge_sb = {}
                for p in range(E // 2):
                    for c in range(NCHUNK):
                        t = cpool.tile([D, 2 * S], dt.bfloat16,
                                       tag=f"edge{p}_{c}")
                        r0 = p * N + c * D
                        nc.sync.dma_start(t[:], edgek.ap()[r0:r0 + D, :])
                        edge_sb[(p, c)] = t
                wmsg_sb = cpool.tile([D, E * D], dt.bfloat16, tag="wmsg")
                nc.sync.dma_start(wmsg_sb[:], wmsg.ap())
                wi_sb = cpool.tile([D, 3 * D], dt.bfloat16, tag="wi")
                nc.sync.dma_start(wi_sb[:], wi.ap())
                wh_sb = cpool.tile([D, 3 * D], dt.bfloat16, tag="wh")
                nc.sync.dma_start(wh_sb[:], wh.ap())
                bias_sb = cpool.tile([D, 3], dt.float32, tag="bias")
                nc.sync.dma_start(bias_sb[:], bias.ap())
                aro_sb = cpool.tile([D, D], dt.bfloat16, tag="aro")
                nc.sync.dma_start(aro_sb[:], aro.ap())
                ident_sb = cpool.tile([D, D], dt.bfloat16, tag="ident")
                nc.sync.dma_start(ident_sb[:], ident.ap())

                hT = spool.tile([D, S], dt.bfloat16, tag="hT")
                nc.sync.dma_start(hT[:], h0t.ap())

                for it in range(ITERS):
                    # ---- full h in node-rows layout [s, d], 16 chunks ----
                    hs = spool.tile([D, N], dt.bfloat16, tag="hs")
                    if it == 0:
                        for c in range(NCHUNK):
                            nc.sync.dma_start(hs[:, c * D:(c + 1) * D],
                                              h0r.ap()[c * D:(c + 1) * D, :])
                    else:
                        agout = dram.tile([N, D], dt.bfloat16, tag="agout")
                        if not skip_coll:
                            nc.gpsimd.collective_compute(
                                "AllGather", mybir.AluOpType.bypass,
                                replica_groups=RG,
                                ins=[agin.opt()], outs=[agout.opt()],
                            )
                            for c in range(NCHUNK):
                                nc.sync.dma_start(hs[:, c * D:(c + 1) * D],
                                                  agout[c * D:(c + 1) * D, :])
                        else:
                            for c in range(NCHUNK):
                                nc.sync.dma_start(hs[:, c * D:(c + 1) * D],
                                                  agin[(c % 2) * D:(c % 2 + 1) * D, :])

                    # ---- agg_e[d, t] = sum_s h[s, d] edge_e[s, t] ----
                    # channels paired 2-per-bank -> 4 full-bank [128, 512]
                    # accumulators, 64 N=512 matmuls in a single pass.
                    aggs = []
                    for q in range(4):
                        ag_t = agg_ps.tile([D, 2 * S], dt.float32,
                                           tag=f"p{q}", name=f"ag_{q}")
                        aggs.append(ag_t)
                    for c in range(NCHUNK):
                        for q in range(4):
                            nc.tensor.matmul(
                                aggs[q][:],
                                hs[:, c * D:(c + 1) * D],
                                edge_sb[(q, c)][:],
                                start=(c == 0), stop=(c == NCHUNK - 1),
                            )
                    agg_bf = []
                    for q in range(4):
                        ab = stpool.tile([D, 2 * S], dt.bfloat16,
                                         tag=f"aggbf{q}")
                        if q % 2 == 0:
                            nc.vector.tensor_copy(ab[:], aggs[q][:])
                        else:
                            nc.scalar.activation(ab[:], aggs[q][:], AF.Copy)
                        agg_bf.append(ab)
                    # ---- msgsT[h, t] = sum_e W_e^T agg_e ----
                    mp = gru_ps.tile([D, S], dt.float32, tag="gru")
                    for e in range(E):
                        nc.tensor.matmul(
                            mp[:], wmsg_sb[:, e * D:(e + 1) * D],
                            agg_bf[e // 2][:, (e % 2) * S:(e % 2 + 1) * S],
                            start=(e == 0), stop=(e == E - 1))
                    msgs_bf = spool.tile([D, S], dt.bfloat16, tag="msgsbf")
                    nc.vector.tensor_copy(msgs_bf[:], mp[:])

                    # ---- GRU (transposed layout, b_msg folded into bias).
                    # gh-half matmuls only need hT/wh: emitted with start=True
                    # so Tile can run them during the AG/agg of this iter;
                    # the gi-half accumulates on top once msgs arrive.
                    new_hT = spool.tile([D, S], dt.bfloat16, tag="hT")
                    gps = []
                    for g in range(3):
                        gp = gru_ps.tile([D, S], dt.float32, tag="gru")
                        nc.tensor.matmul(gp[:], wh_sb[:, g * D:(g + 1) * D],
                                         hT[:], start=True, stop=(g == 2))
                        gps.append(gp)
                    gate = []
                    for g in range(2):
                        nc.tensor.matmul(gps[g][:],
                                         wi_sb[:, g * D:(g + 1) * D],
                                         msgs_bf[:], start=False, stop=True)
                        gs = stpool.tile([D, S], dt.float32, tag=f"g{g}")
                        nc.scalar.activation(gs[:], gps[g][:], AF.Sigmoid,
                                             bias=bias_sb[:, g:g + 1])
                        gate.append(gs)
                    r_g, z_g = gate

                    hnp = gps[2]
                    inp = gru_ps.tile([D, S], dt.float32, tag="gru")
                    nc.tensor.matmul(inp[:], wi_sb[:, 2 * D:3 * D], msgs_bf[:],
                                     start=True, stop=True)
                    t1 = stpool.tile([D, S], dt.float32, tag="t1")
                    nc.vector.tensor_mul(t1[:], r_g[:], hnp[:])
                    t2 = stpool.tile([D, S], dt.float32, tag="t2")
                    nc.vector.tensor_add(t2[:], t1[:], inp[:])
                    n_sb = stpool.tile([D, S], dt.float32, tag="n")
                    nc.scalar.activation(n_sb[:], t2[:], AF.Tanh,
                                         bias=bias_sb[:, 2:3])
                    d1 = stpool.tile([D, S], dt.float32, tag="d1")
                    nc.vector.tensor_sub(d1[:], hT[:], n_sb[:])
                    d2 = stpool.tile([D, S], dt.float32, tag="d2")
                    nc.vector.tensor_mul(d2[:], z_g[:], d1[:])
                    nc.vector.tensor_add(new_hT[:], n_sb[:], d2[:])
                    hT = new_hT

                    # ---- h_new -> node-rows, staged for the next AllGather --
                    agin = dram.tile([S, D], dt.bfloat16, tag="agin")
                    for k in range(2):
                        tp = agg_ps.tile([D, D], dt.bfloat16, tag=f"p{k}")
                        nc.tensor.transpose(tp[:], hT[:, k * D:(k + 1) * D],
                                            ident_sb[:])
                        hr = stpool.tile([D, D], dt.bfloat16, tag=f"hr{k}")
                        nc.vector.tensor_copy(hr[:], tp[:])
                        nc.sync.dma_start(agin[k * D:(k + 1) * D, :], hr[:])

                # ---- readout: AG final h, X-bar transpose to [D, N] ----
                agout = dram.tile([N, D], dt.bfloat16, tag="agout")
                if not skip_coll:
                    nc.gpsimd.collective_compute(
                        "AllGather", mybir.AluOpType.bypass, replica_groups=RG,
                        ins=[agin.opt()], outs=[agout.opt()],
                    )
                else:
                    nc.sync.dma_start(agout[0:S, :], agin[:])
                hTf = spool.tile([D, N], dt.bfloat16, tag="hTf")
                nc.sync.dma_start(hTf[:], agout[:], transpose=True)

                hap = agg_ps.tile([D, S], dt.float32, tag="p3")
                nc.tensor.matmul(hap[:], aro_sb[:], hT[:], start=True, stop=True)
                hA_bf = spool.tile([D, S], dt.bfloat16, tag="hA")
                nc.vector.tensor_copy(hA_bf[:], hap[:])

                for isub in range(2):
                    for jc in range(N_CORES):
                        lp = gru_ps.tile([D, S], dt.float32, tag="gru")
                        nc.tensor.matmul(lp[:],
                                         hA_bf[:, isub * D:(isub + 1) * D],
                                         hTf[:, jc * S:(jc + 1) * S],
                                         start=True, stop=True)
                        ost = stpool.tile([D, S], dt.float32, tag="ost")
                        nc.vector.tensor_copy(ost[:], lp[:])
                        nc.sync.dma_start(
                            out.ap()[isub * D:(isub + 1) * D,
                                     jc * S:(jc + 1) * S],
                            ost[:])

    nc.compile()
    return nc


def make_in_maps(node_embeddings, edge_embeddings, W_msg, b_msg, Wi, Wh,
                 b_gru, A_readout):
    bf16 = ml_dtypes.bfloat16
    wmsg_h = np.ascontiguousarray(
        W_msg.transpose(1, 0, 2).reshape(D, E * D)).astype(bf16)
    wi_b = np.ascontiguousarray(Wi).astype(bf16)
    wh_b = np.ascontiguousarray(Wh).astype(bf16)
    b_eff = (b_msg.astype(np.float64) @ Wi.astype(np.float64)
             + b_gru.astype(np.float64)).astype(np.float32)
    bias_h = np.ascontiguousarray(b_eff.reshape(3, D).T)
    aro_b = np.ascontiguousarray(A_readout).astype(bf16)
    ident_h = np.eye(D, dtype=np.float32).astype(bf16)
    h0r_h = np.ascontiguousarray(node_embeddings).astype(bf16)

    in_maps = []
    for k in range(N_CORES):
        sl = slice(k * S, (k + 1) * S)
        # edge[:, t_own, :] -> [(pair, s), (e%2, t_own)]: channels 2p and
        # 2p+1 side by side so one [128, 512] matmul accumulates both.
        ek = np.ascontiguousarray(
            edge_embeddings[:, sl, :].reshape(N, S, E // 2, 2)
            .transpose(2, 0, 3, 1).reshape(E // 2 * N, 2 * S)
        ).astype(bf16)
        h0t_h = np.ascontiguousarray(node_embeddings[sl].T).astype(bf16)
        in_maps.append({
            "edgek": ek, "h0t": h0t_h, "h0r": h0r_h, "wmsg": wmsg_h,
            "wi": wi_b, "wh": wh_b, "bias": bias_h, "aro": aro_b,
            "ident": ident_h,
        })
    return in_maps


_cache = {}


def kernel(node_embeddings, edge_embeddings, W_msg, b_msg, Wi, Wh, b_gru,
           A_readout):
    if "nc" not in _cache:
        _cache["nc"] = build_nc(reps=1)
    nc = _cache["nc"]
    in_maps = make_in_maps(node_embeddings, edge_embeddings, W_msg, b_msg,
                           Wi, Wh, b_gru, A_readout)
    res = bass_utils.run_bass_kernel_spmd(
        nc, in_maps, core_ids=list(range(N_CORES)))
    return np.concatenate([res.results[k]["out"] for k in range(N_CORES)],
                          axis=0)


# revision 3
# speedup vs baseline: 1.1297x; 1.0347x over previous
"""GGNN message passing + bilinear readout on 8 TRN2 NeuronCores (v3b: destination-sharded + AllGather).

Destination-sharded 1D node parallelism with the "agg" contraction order:

  core k owns destinations t_k = [256k, 256k+256).
  - edge shard edge[:, t_k, :] resident in SBUF as 8(e) x 16(s-chunk)
    tiles of [128 senders, 256 dst] bf16 (8 MiB).
  - per iter: all-gather h (64 KB per core, node-rows [N, D]) — the
    cheapest collective primitive (~5 us at 8 cores) — then locally:
        agg_e[d, t]  = sum_s h[s, d] * edge_e[s, t]      (128 MMs, PSUM acc)
        msgsT[h, t]  = sum_e W_e[d, h]^T-contracted agg_e  (8 MMs)
        h_new        = GRU(msgsT, hT_own)                  (as v1)
    and 2 PE transposes turn h_new^T into node-rows for the next AG.
  - h0 is supplied replicated host-side, so only 7 iteration AGs + 1
    readout AG run. The readout AG output [N, D] is DMA-transposed
    (X-bar) into hT-full [D, N] for the bilinear logits rows.

No collective ReduceScatter of 512 KB partials (v1: ~21 us/round), no
remote-DMA descriptor generation (~10 us/prep on Q7), no critical
sections — everything stays on Tile's scheduler so DMA/collective/compute
overlap freely.
"""

import sys

for _p in ("/opt/trn_rl_repo",):
    if _p not in sys.path:
        sys.path.insert(0, _p)

import numpy as np
import ml_dtypes

import concourse.bacc as bacc
import concourse.tile as tile
import concourse.mybir as mybir
from concourse import bass_utils

dt = mybir.dt
AF = mybir.ActivationFunctionType

N_CORES = 8
N = 2048
D = 128
E = 8
ITERS = 8
S = N // N_CORES  # 256
NCHUNK = N // D   # 16 sender chunks
RG = [list(range(N_CORES))]


def build_nc(reps=1, skip_coll=False):
    nc = bacc.Bacc("TRN2", target_bir_lowering=False, debug=False,
                   num_devices=N_CORES)

    # edge shard, sender-chunk-major: [(e, s_chunk) x 128, t_own]
    edgek = nc.dram_tensor("edgek", [E // 2 * N, 2 * S], dt.bfloat16,
                           kind="ExternalInput")
    h0t = nc.dram_tensor("h0t", [D, S], dt.bfloat16, kind="ExternalInput")
    h0r = nc.dram_tensor("h0r", [N, D], dt.bfloat16, kind="ExternalInput")
    wmsg = nc.dram_tensor("wmsg", [D, E * D], dt.bfloat16, kind="ExternalInput")
    wi = nc.dram_tensor("wi", [D, 3 * D], dt.bfloat16, kind="ExternalInput")
    wh = nc.dram_tensor("wh", [D, 3 * D], dt.bfloat16, kind="ExternalInput")
    bias = nc.dram_tensor("bias", [D, 3], dt.float32, kind="ExternalInput")
    aro = nc.dram_tensor("aro", [D, D], dt.bfloat16, kind="ExternalInput")
    ident = nc.dram_tensor("ident", [D, D], dt.bfloat16, kind="ExternalInput")
    out = nc.dram_tensor("out", [S, N], dt.float32, kind="ExternalOutput")

    with tile.TileContext(nc) as tc:
        with (
            tc.tile_pool(name="const", bufs=1) as cpool,
            tc.tile_pool(name="sb", bufs=2) as spool,
            tc.tile_pool(name="stage", bufs=2) as stpool,
            tc.tile_pool(name="agg_ps", bufs=1, space="PSUM") as agg_ps,
            tc.tile_pool(name="gru_ps", bufs=4, space="PSUM") as gru_ps,
            tc.tile_pool(name="dram", bufs=2, space="DRAM") as dram,
        ):
            for rep in range(reps):
                # ---- resident constants ----
                edge_sb = {}
                for p in range(E // 2):
                    for c in range(NCHUNK):
                        t = cpool.tile([D, 2 * S], dt.bfloat16,
                                       tag=f"edge{p}_{c}")
                        r0 = p * N + c * D
                        nc.sync.dma_start(t[:], edgek.ap()[r0:r0 + D, :])
                        edge_sb[(p, c)] = t
                wmsg_sb = cpool.tile([D, E * D], dt.bfloat16, tag="wmsg")
                nc.sync.dma_start(wmsg_sb[:], wmsg.ap())
                wi_sb = cpool.tile([D, 3 * D], dt.bfloat16, tag="wi")
                nc.sync.dma_start(wi_sb[:], wi.ap())
                wh_sb = cpool.tile([D, 3 * D], dt.bfloat16, tag="wh")
                nc.sync.dma_start(wh_sb[:], wh.ap())
                bias_sb = cpool.tile([D, 3], dt.float32, tag="bias")
                nc.sync.dma_start(bias_sb[:], bias.ap())
                aro_sb = cpool.tile([D, D], dt.bfloat16, tag="aro")
                nc.sync.dma_start(aro_sb[:], aro.ap())
                ident_sb = cpool.tile([D, D], dt.bfloat16, tag="ident")
                nc.sync.dma_start(ident_sb[:], ident.ap())

                hT = spool.tile([D, S], dt.bfloat16, tag="hT")
                nc.sync.dma_start(hT[:], h0t.ap())

                for it in range(ITERS):
                    # ---- full h in node-rows layout [s, d], 16 chunks ----
                    hs = spool.tile([D, N], dt.bfloat16, tag="hs")
                    if it == 0:
                        for c in range(NCHUNK):
                            nc.sync.dma_start(hs[:, c * D:(c + 1) * D],
                                              h0r.ap()[c * D:(c + 1) * D, :])
                    else:
                        # two pipelined half-gathers: AG_A's input (own first
                        # 128 nodes) is staged before the second GRU half is
                        # transposed, so AG_A flies while B is still staged,
                        # and first-half agg chunks start under AG_B.
                        for half in range(2):
                            agout = dram.tile([N // 2, D], dt.bfloat16,
                                              tag=f"agout{half}",
                                              name=f"agout{half}")
                            if not skip_coll:
                                nc.gpsimd.collective_compute(
                                    "AllGather", mybir.AluOpType.bypass,
                                    replica_groups=RG,
                                    ins=[agins[half].opt()],
                                    outs=[agout.opt()],
                                )
                            for c in range(8):
                                nc.sync.dma_start(
                                    hs[:, (half * 8 + c) * D:
                                          (half * 8 + c + 1) * D],
                                    agout[c * D:(c + 1) * D, :])

                    # ---- agg_e[d, t] = sum_s h[s, d] edge_e[s, t] ----
                    # channels paired 2-per-bank -> 4 full-bank [128, 512]
                    # accumulators, 64 N=512 matmuls in a single pass.
                    aggs = []
                    for q in range(4):
                        ag_t = agg_ps.tile([D, 2 * S], dt.float32,
                                           tag=f"p{q}", name=f"ag_{q}")
                        aggs.append(ag_t)
                    for c in range(NCHUNK):
                        for q in range(4):
                            nc.tensor.matmul(
                                aggs[q][:],
                                hs[:, c * D:(c + 1) * D],
                                edge_sb[(q, c)][:],
                                start=(c == 0), stop=(c == NCHUNK - 1),
                            )
                    agg_bf = []
                    for q in range(4):
                        ab = stpool.tile([D, 2 * S], dt.bfloat16,
                                         tag=f"aggbf{q}")
                        if q % 2 == 0:
                            nc.vector.tensor_copy(ab[:], aggs[q][:])
                        else:
                            nc.scalar.activation(ab[:], aggs[q][:], AF.Copy)
                        agg_bf.append(ab)
                    # ---- msgsT[h, t] = sum_e W_e^T agg_e ----
                    mp = gru_ps.tile([D, S], dt.float32, tag="gru")
                    for e in range(E):
                        nc.tensor.matmul(
                            mp[:], wmsg_sb[:, e * D:(e + 1) * D],
                            agg_bf[e // 2][:, (e % 2) * S:(e % 2 + 1) * S],
                            start=(e == 0), stop=(e == E - 1))
                    msgs_bf = spool.tile([D, S], dt.bfloat16, tag="msgsbf")
                    nc.vector.tensor_copy(msgs_bf[:], mp[:])

                    # ---- GRU (transposed layout, b_msg folded into bias).
                    # gh-half matmuls only need hT/wh: emitted with start=True
                    # so Tile can run them during the AG/agg of this iter;
                    # the gi-half accumulates on top once msgs arrive.
                    new_hT = spool.tile([D, S], dt.bfloat16, tag="hT")
                    gps = []
                    for g in range(3):
                        gp = gru_ps.tile([D, S], dt.float32, tag="gru")
                        nc.tensor.matmul(gp[:], wh_sb[:, g * D:(g + 1) * D],
                                         hT[:], start=True, stop=(g == 2))
                        gps.append(gp)
                    gate = []
                    for g in range(2):
                        nc.tensor.matmul(gps[g][:],
                                         wi_sb[:, g * D:(g + 1) * D],
                                         msgs_bf[:], start=False, stop=True)
                        gs = stpool.tile([D, S], dt.float32, tag=f"g{g}")
                        nc.scalar.activation(gs[:], gps[g][:], AF.Sigmoid,
                                             bias=bias_sb[:, g:g + 1])
                        gate.append(gs)
                    r_g, z_g = gate

                    hnp = gps[2]
                    inp = gru_ps.tile([D, S], dt.float32, tag="gru")
                    nc.tensor.matmul(inp[:], wi_sb[:, 2 * D:3 * D], msgs_bf[:],
                                     start=True, stop=True)
                    t1 = stpool.tile([D, S], dt.float32, tag="t1")
                    nc.vector.tensor_mul(t1[:], r_g[:], hnp[:])
                    t2 = stpool.tile([D, S], dt.float32, tag="t2")
                    nc.vector.tensor_add(t2[:], t1[:], inp[:])
                    n_sb = stpool.tile([D, S], dt.float32, tag="n")
                    nc.scalar.activation(n_sb[:], t2[:], AF.Tanh,
                                         bias=bias_sb[:, 2:3])
                    d1 = stpool.tile([D, S], dt.float32, tag="d1")
                    nc.vector.tensor_sub(d1[:], hT[:], n_sb[:])
                    d2 = stpool.tile([D, S], dt.float32, tag="d2")
                    nc.vector.tensor_mul(d2[:], z_g[:], d1[:])
                    nc.vector.tensor_add(new_hT[:], n_sb[:], d2[:])
                    hT = new_hT

                    # ---- h_new -> node-rows, staged for the half-AGs ----
                    agins = []
                    for k in range(2):
                        agin = dram.tile([S // 2, D], dt.bfloat16,
                                         tag=f"agin{k}", name=f"agin{k}")
                        tp = agg_ps.tile([D, D], dt.bfloat16, tag=f"p{k}")
                        nc.tensor.transpose(tp[:], hT[:, k * D:(k + 1) * D],
                                            ident_sb[:])
                        hr = stpool.tile([D, D], dt.bfloat16, tag=f"hr{k}")
                        nc.vector.tensor_copy(hr[:], tp[:])
                        nc.sync.dma_start(agin[:], hr[:])
                        agins.append(agin)

                # ---- readout: AG final h, X-bar transpose to [D, N] ----
                hTf = spool.tile([D, N], dt.bfloat16, tag="hTf")
                for half in range(2):
                    agout = dram.tile([N // 2, D], dt.bfloat16,
                                      tag=f"agout{half}", name=f"agout{half}")
                    if not skip_coll:
                        nc.gpsimd.collective_compute(
                            "AllGather", mybir.AluOpType.bypass,
                            replica_groups=RG,
                            ins=[agins[half].opt()], outs=[agout.opt()],
                        )
                    nc.sync.dma_start(
                        hTf[:, half * (N // 2):(half + 1) * (N // 2)],
                        agout[:], transpose=True)

                hap = agg_ps.tile([D, S], dt.float32, tag="p3")
                nc.tensor.matmul(hap[:], aro_sb[:], hT[:], start=True, stop=True)
                hA_bf = spool.tile([D, S], dt.bfloat16, tag="hA")
                nc.vector.tensor_copy(hA_bf[:], hap[:])

                for isub in range(2):
                    for jc in range(N_CORES):
                        lp = gru_ps.tile([D, S], dt.float32, tag="gru")
                        nc.tensor.matmul(lp[:],
                                         hA_bf[:, isub * D:(isub + 1) * D],
                                         hTf[:, jc * S:(jc + 1) * S],
                                         start=True, stop=True)
                        ost = stpool.tile([D, S], dt.float32, tag="ost")
                        nc.vector.tensor_copy(ost[:], lp[:])
                        nc.sync.dma_start(
                            out.ap()[isub * D:(isub + 1) * D,
                                     jc * S:(jc + 1) * S],
                            ost[:])

    nc.compile()
    return nc


def make_in_maps(node_embeddings, edge_embeddings, W_msg, b_msg, Wi, Wh,
                 b_gru, A_readout):
    bf16 = ml_dtypes.bfloat16
    wmsg_h = np.ascontiguousarray(
        W_msg.transpose(1, 0, 2).reshape(D, E * D)).astype(bf16)
    wi_b = np.ascontiguousarray(Wi).astype(bf16)
    wh_b = np.ascontiguousarray(Wh).astype(bf16)
    b_eff = (b_msg.astype(np.float64) @ Wi.astype(np.float64)
             + b_gru.astype(np.float64)).astype(np.float32)
    bias_h = np.ascontiguousarray(b_eff.reshape(3, D).T)
    aro_b = np.ascontiguousarray(A_readout).astype(bf16)
    ident_h = np.eye(D, dtype=np.float32).astype(bf16)
    h0r_h = np.ascontiguousarray(
        node_embeddings.reshape(NCHUNK, D, D)[CHUNK_PERM]
        .reshape(N, D)).astype(bf16)

    in_maps = []
    for k in range(N_CORES):
        sl = slice(k * S, (k + 1) * S)
        # edge[:, t_own, :] -> [(pair, s), (e%2, t_own)]: channels 2p and
        # 2p+1 side by side so one [128, 512] matmul accumulates both.
        # Sender chunks reordered to the half-AG arrival order: all cores'
        # first 128 nodes (even natural chunks), then all second halves.
        ek = (edge_embeddings[:, sl, :].reshape(N, S, E // 2, 2)
              .transpose(2, 0, 3, 1).reshape(E // 2, NCHUNK, D, 2 * S))
        ek = np.ascontiguousarray(
            ek[:, CHUNK_PERM].reshape(E // 2 * N, 2 * S)).astype(bf16)
        h0t_h = np.ascontiguousarray(node_embeddings[sl].T).astype(bf16)
        in_maps.append({
            "edgek": ek, "h0t": h0t_h, "h0r": h0r_h, "wmsg": wmsg_h,
            "wi": wi_b, "wh": wh_b, "bias": bias_h, "aro": aro_b,
            "ident": ident_h,
        })
    return in_maps


# natural sender-chunk n at half-AG position: evens (cores' first 128
# nodes) land in chunks 0-7, odds in 8-15.
CHUNK_PERM = [2 * c for c in range(8)] + [2 * c + 1 for c in range(8)]
# hTf/logits column j corresponds to node COL_NODE[j]
COL_NODE = np.concatenate([np.arange(c * D, (c + 1) * D)
                           for c in CHUNK_PERM])


def unpermute_out(results):
    logits = np.empty((N, N), np.float32)
    full = np.concatenate([results[k]["out"] for k in range(N_CORES)], axis=0)
    logits[:, COL_NODE] = full
    return logits


_cache = {}


def kernel(node_embeddings, edge_embeddings, W_msg, b_msg, Wi, Wh, b_gru,
           A_readout):
    if "nc" not in _cache:
        _cache["nc"] = build_nc(reps=1)
    nc = _cache["nc"]
    in_maps = make_in_maps(node_embeddings, edge_embeddings, W_msg, b_msg,
                           Wi, Wh, b_gru, A_readout)
    res = bass_utils.run_bass_kernel_spmd(
        nc, in_maps, core_ids=list(range(N_CORES)))
    return unpermute_out(res.results)
